# revision 50
# baseline (speedup 1.0000x reference)
"""Trainium2 Bass kernel for C2C attention.

Computes, for x:(B,C,T)=(32,64,30000) f32:
    desc = mean(x, axis=2)                       # (B,C)
    q = desc*Wq + bq ; k = desc*Wk + bk          # (B,C,D), D=64
    attn = softmax(q @ k^T / sqrt(D))            # (B,C,C)
    out = x + alpha * attn @ x

Sharding: pure data parallel over batch, 4 batches per core on 8 cores.
Each core stacks its 4 batches as 2 "pairs" of 128 SBUF partitions
(2 batches x 64 channels); a block-diagonal 128x128 stationary matrix
computes both batches of a pair in one matmul pass.

Transport is bf16 both ways (host rounds x to bf16; host expands the
bf16 result back to f32) which halves HBM traffic; all computation runs
on device.  Residual + softmax normalization are folded into the matmul
pipeline:

    M^T = diag(sumexp/alpha) + exp(logits)^T    (stationary, bf16)
    out_row_c = (alpha/sumexp_c) * (M x)_c      (scale applied by the
                                                 PSUM->SBUF copy)

Engine plan (GPSIMD cannot touch PSUM, so):
  - DVE: bf16 fold-tree segment reductions (2x DVE mode) + a share of
    the PSUM->SBUF copies once its reduction queue drains
  - ACT: reduction assists on pair0 (activation accum_out), the small
    attention chain, most PSUM->SBUF copies
  - GPSIMD: first-level folds for pair1 reductions, stationary-matrix
    assembly (all SBUF->SBUF)
  - PE: streaming matmul, kept continuously busy so it up-clocks
"""

import os

import numpy as np
import ml_dtypes

import concourse.bass as bass
import concourse.tile as tile
from concourse import bacc, mybir
from concourse.bass_utils import run_bass_kernel_spmd


B, C, T, D = 32, 64, 30000, 64
N_CORES = 8
BPC = B // N_CORES          # batches per core = 4
PAIRS = BPC // 2            # 2
ROWS = BPC * C              # 256 rows of (row, T) per core
SEG = 6000                  # columns per DMA segment (12000B/row descriptors
                            # keep the DMA engines at full ~425GB/s; 6000B
                            # rows measured only ~300GB/s)
NSEG = T // SEG             # 5
CHUNK = 500                 # matmul moving free dim (<=512, fits PSUM bank)
GROUP = 2                   # chunks per PSUM tile (2 banks) -> 1000-col copies
NGRP = SEG // (CHUNK * GROUP)   # 6 groups per segment

F32 = mybir.dt.float32
BF16 = mybir.dt.bfloat16
AX = mybir.AxisListType
AF = mybir.ActivationFunctionType
ALU = mybir.AluOpType

# packed constants layout, one (128, 515) f32 block:
#   [:, 0:128]    identity(128)
#   [:, 128:129]  alpha broadcast
#   [0:2, 129:257]   [Wq/(8T); bq/8 | Wk/T; bk]  (stacked q|k weight rows)
#   [0:2, 257:385]   qk-matmul rhs init: row0 = 0 (sums placeholder), row1 = 1
#   [:, 385:513]  zeros -> attn scratch (off-diagonal blocks must stay 0)
#   [:, 513:514]  ones column (unused)
#   [:, 514:515]  1/alpha broadcast (unused)
#   [:, 515:516]  ln(1/alpha) broadcast (exp bias for the sumexp accum)
CONST_COLS = 516

# pair0 segments whose reduction runs on ACT (rest on DVE — including the
# tail segment, so the last-arriving data is reduced by the faster engine)
P0_ACT_SEGS = {1, 3}
# pair1 segments folded on GPSIMD (rest fully on DVE)
P1_GP_SEGS = (0, 2)
# copy groups assigned to DVE: late groups of pair0, alternating for pair1
P0_DVE_GROUPS = {15, 17, 19, 21, 23, 25, 27, 29}
P1_DVE_GROUPS = {3, 5, 7, 9, 11, 13, 15, 17, 19, 21, 23, 25, 27, 29}


def build_bass() -> bass.Bass:
    nc = bacc.Bacc()

    x = nc.dram_tensor("x", [ROWS, T], BF16, kind="ExternalInput")
    out = nc.dram_tensor("out", [ROWS, T], BF16, kind="ExternalOutput")
    consts_d = nc.dram_tensor("consts", [128, CONST_COLS], F32,
                              kind="ExternalInput")

    with tile.TileContext(nc) as tc, \
            tc.tile_pool(name="consts", bufs=1) as consts, \
            tc.tile_pool(name="pairbuf", bufs=2) as pairbuf, \
            tc.tile_pool(name="fold", bufs=2) as fold, \
            tc.tile_pool(name="xsegs", bufs=PAIRS * NSEG) as xsegs, \
            tc.tile_pool(name="psmm", bufs=3, space="PSUM") as psmm, \
            tc.tile_pool(name="pssm", bufs=2, space="PSUM") as pssm:

        cblk = consts.tile([128, CONST_COLS], F32)
        nc.sync.dma_start(out=cblk, in_=consts_d[:, :])
        ident = cblk[:, 0:128]
        alpha_bc = cblk[:, 128:129]
        wqk2 = cblk[0:2, 129:257]
        rhs_qk = cblk[0:2, 257:385]
        attn = cblk[:, 385:513]
        ln_invalpha = cblk[:, 515:516]
        scratch = consts.tile([128, 1], F32)
        # pre-load the ACT exp table off the critical path
        nc.scalar.activation(out=scratch, in_=alpha_bc, func=AF.Exp)
        # full-size scratch sink for ACT accumulate-reductions
        accsink = consts.tile([128, SEG], BF16)

        xs = [[None] * NSEG for _ in range(PAIRS)]
        partials = [None] * PAIRS
        lhsT = [None] * PAIRS
        arec = [None] * PAIRS
        sums = [None] * PAIRS
        sumexp = [None] * PAIRS
        diags = [None] * PAIRS

        def emit_load(p, s):
            xt = xsegs.tile([128, SEG], BF16, tag="xseg")
            xs[p][s] = xt
            nc.sync.dma_start(
                out=xt,
                in_=x[p * 128:(p + 1) * 128, s * SEG:(s + 1) * SEG],
            )

        def emit_reduce_dve(p, s):
            xt = xs[p][s]
            h = fold.tile([128, SEG // 2], BF16, tag="h")
            nc.vector.tensor_add(out=h, in0=xt[:, 0:SEG // 2],
                                 in1=xt[:, SEG // 2:SEG])
            q = fold.tile([128, SEG // 4], BF16, tag="q")
            nc.vector.tensor_add(out=q, in0=h[:, 0:SEG // 4],
                                 in1=h[:, SEG // 4:SEG // 2])
            w = fold.tile([128, SEG // 8], BF16, tag="w")
            nc.vector.tensor_add(out=w, in0=q[:, 0:SEG // 8],
                                 in1=q[:, SEG // 8:SEG // 4])
            nc.vector.reduce_sum(out=partials[p][:, s:s + 1], in_=w, axis=AX.X)

        def emit_reduce_act(p, s):
            # ACT-assisted reduction: copy into a scratch sink, accumulate
            # the row sum as a side effect
            nc.scalar.activation(out=accsink, in_=xs[p][s], func=AF.Copy,
                                 accum_out=partials[p][:, s:s + 1])

        gp_folds = {}

        def emit_gp_fold(p, s):
            # both fold levels on GPSIMD (SBUF->SBUF); DVE only reduces
            xt = xs[p][s]
            gh = fold.tile([128, SEG // 2], BF16, tag="gh")
            nc.gpsimd.tensor_add(out=gh, in0=xt[:, 0:SEG // 2],
                                 in1=xt[:, SEG // 2:SEG])
            gq = fold.tile([128, SEG // 4], BF16, tag="gq")
            nc.gpsimd.tensor_add(out=gq, in0=gh[:, 0:SEG // 4],
                                 in1=gh[:, SEG // 4:SEG // 2])
            gp_folds[(p, s)] = gq

        def emit_dve_finish(p, s):
            gq = gp_folds[(p, s)]
            w = fold.tile([128, SEG // 8], BF16, tag="w")
            nc.vector.tensor_add(out=w, in0=gq[:, 0:SEG // 8],
                                 in1=gq[:, SEG // 8:SEG // 4])
            nc.vector.reduce_sum(out=partials[p][:, s:s + 1], in_=w, axis=AX.X)

        def alloc_partials(p):
            part = pairbuf.tile([128, NSEG], F32, tag="partial")
            partials[p] = part

        def emit_sums(p):
            sm = pairbuf.tile([128, 1], F32, tag="sums")
            nc.vector.reduce_sum(out=sm, in_=partials[p], axis=AX.X)
            sums[p] = sm

        def emit_smalls_head(p):
            # transpose total sums to a row: (1,128)
            srow_ps = pssm.tile([1, 128], F32, tag="ps_small")
            nc.tensor.transpose(out=srow_ps, in_=sums[p], identity=ident)
            nc.scalar.copy(out=rhs_qk[0:1, :], in_=srow_ps)
            # [qT; kT] = [wq'|wk']^T @ [sums_row; ones] : (128, 2C)
            qk_ps = pssm.tile([128, 2 * C], F32, tag="ps_small")
            nc.tensor.matmul(out=qk_ps, lhsT=wqk2, rhs=rhs_qk,
                             start=True, stop=True)
            qT = pairbuf.tile([D, 2 * C], F32, tag="qT")
            nc.scalar.copy(out=qT, in_=qk_ps[0:64, :])
            kT = pairbuf.tile([D, 2 * C], F32, tag="kT")
            nc.scalar.copy(out=kT, in_=qk_ps[64:128, :])
            # transposed logits lgT[e,c] = k_e . q_c, and plain logits
            # lg[c,e] (back-to-back PE ops, no extra hop)
            lg_ps = pssm.tile([128, 128], F32, tag="ps_small")
            nc.tensor.matmul(out=lg_ps, lhsT=qT, rhs=kT, start=True, stop=True)
            lgT_ps = pssm.tile([128, 128], F32, tag="ps_small")
            nc.tensor.matmul(out=lgT_ps, lhsT=kT, rhs=qT, start=True, stop=True)
            # exp of each diagonal block into the zeroed attn scratch; the
            # second (throwaway-output) exp over the plain logits accumulates
            # se_c = sum_e exp(lg[c,e] + ln(1/alpha)) = sumexp_c / alpha, so
            # reciprocal(se) = alpha/sumexp — exactly the copy scale
            se = pairbuf.tile([128, 1], F32, tag="sumexp")
            for h in range(2):
                r = slice(h * 64, h * 64 + 64)
                nc.scalar.activation(out=attn[r, r], in_=lgT_ps[r, r],
                                     func=AF.Exp)
                nc.scalar.activation(out=accsink[r, 0:64], in_=lg_ps[r, r],
                                     func=AF.Exp, bias=ln_invalpha[r, :],
                                     accum_out=se[r, :])
            sumexp[p] = se
            # diagonal part of the stationary matrix: diag(sumexp/alpha)
            diag = pairbuf.tile([128, 128], F32, tag="diag")
            nc.scalar.activation(out=diag, in_=ident, func=AF.Copy, scale=se)
            diags[p] = diag

        def emit_lt_dve(p):
            # stationary matrix M^T = diag(sumexp/alpha) + exp(logits)^T
            lt = pairbuf.tile([128, 128], BF16, tag="lhsT")
            nc.vector.tensor_add(out=lt, in0=diags[p], in1=attn)
            lhsT[p] = lt

        def emit_recip(p):
            # alpha/sumexp scale for the PSUM->SBUF copies
            ar = pairbuf.tile([128, 1], F32, tag="arec")
            nc.vector.reciprocal(out=ar, in_=sumexp[p])
            arec[p] = ar

        def emit_compute(p, segs, dve_groups, split_last=False):
            for s in segs:
                xt = xs[p][s]
                for g in range(NGRP):
                    gi = s * NGRP + g
                    mm = psmm.tile([128, GROUP, 512], F32, tag="mm")
                    base = g * GROUP * CHUNK
                    for j in range(GROUP):
                        nc.tensor.matmul(
                            out=mm[:, j, 0:CHUNK],
                            lhsT=lhsT[p],
                            rhs=xt[:, base + j * CHUNK: base + (j + 1) * CHUNK],
                            start=True, stop=True,
                        )
                    dst = xt[:, base: base + GROUP * CHUNK].rearrange(
                        "p (a c) -> p a c", a=GROUP)
                    if gi in dve_groups:
                        nc.vector.tensor_scalar(out=dst, in0=mm[:, :, 0:CHUNK],
                                                scalar1=arec[p], scalar2=None,
                                                op0=ALU.mult)
                    else:
                        nc.scalar.activation(out=dst, in_=mm[:, :, 0:CHUNK],
                                             func=AF.Copy, scale=arec[p])
                    if split_last and s == segs[-1] and g % 2 == 1:
                        # final segment: drain per pair-of-groups so the last
                        # DMA is small and the tail is short
                        pb = (g - 1) * GROUP * CHUNK
                        nc.sync.dma_start(
                            out=out[p * 128:(p + 1) * 128,
                                    s * SEG + pb: s * SEG + pb
                                    + 2 * GROUP * CHUNK],
                            in_=xt[:, pb: pb + 2 * GROUP * CHUNK],
                        )
                if not (split_last and s == segs[-1]):
                    nc.sync.dma_start(
                        out=out[p * 128:(p + 1) * 128,
                                s * SEG:(s + 1) * SEG],
                        in_=xt,
                    )

        # ---- schedule ----
        alloc_partials(0)
        alloc_partials(1)
        for s in range(NSEG):
            emit_load(0, s)
            if s in P0_ACT_SEGS:
                emit_reduce_act(0, s)
            else:
                emit_reduce_dve(0, s)
        for s in range(NSEG):
            emit_load(1, s)
        emit_sums(0)
        emit_smalls_head(0)
        # pair1 reductions start while the pair0 attention chain finishes;
        # GPSIMD takes the first fold level on early segments so DVE can
        # also pick up part of the pair0 copy load later.  The pair0
        # stationary-matrix add + reciprocal slot into the DVE stream
        # between segment finishes, right when their inputs appear.
        # GPSIMD folds its segments sequentially from first arrival; the DVE
        # stream runs its own independent fold-trees interleaved with the
        # pair0 stationary-matrix add, the copy-scale reciprocal, and the
        # finish-reduces of GPSIMD's segments (ordered so DVE never
        # head-of-line blocks on the slower GPSIMD queue)
        for s in P1_GP_SEGS:
            emit_gp_fold(1, s)
        emit_reduce_dve(1, 1)
        emit_lt_dve(0)
        emit_recip(0)
        emit_reduce_dve(1, 3)
        emit_dve_finish(1, 0)
        emit_reduce_dve(1, 4)
        emit_dve_finish(1, 2)
        emit_sums(1)
        emit_compute(0, range(0, 4), P0_DVE_GROUPS)
        emit_smalls_head(1)
        emit_recip(1)
        emit_lt_dve(1)
        emit_compute(0, range(4, NSEG), P0_DVE_GROUPS)
        emit_compute(1, range(NSEG), P1_DVE_GROUPS, split_last=True)

    nc.compile()
    return nc


def _host_inputs(x, Wq, bq, Wk, bk, Wv, bv, alpha):
    """Build per-core input maps. Scale folding:
    logits[c,e] = (q[c]/8) . k[e],  q/8 = (Wq/(8T))*sums + bq/8, k = (Wk/T)*sums + bk
    """
    x = np.asarray(x, dtype=np.float32).astype(ml_dtypes.bfloat16)
    cb = np.zeros((128, CONST_COLS), dtype=np.float32)
    cb[:, 0:128] = np.eye(128, dtype=np.float32)
    cb[:, 128] = np.float32(alpha)
    cb[0, 129:193] = np.asarray(Wq)[:, 0] / (8.0 * T)
    cb[1, 129:193] = np.asarray(bq) / 8.0
    cb[0, 193:257] = np.asarray(Wk)[:, 0] / T
    cb[1, 193:257] = np.asarray(bk)
    cb[1, 257:385] = 1.0
    cb[:, 513] = 1.0
    cb[:, 514] = 1.0 / np.float32(alpha)
    cb[:, 515] = np.log(1.0 / np.float64(alpha)).astype(np.float32)
    in_maps = []
    for c in range(N_CORES):
        shard = x[c * BPC:(c + 1) * BPC].reshape(ROWS, T)
        in_maps.append({
            "x": np.ascontiguousarray(shard),
            "consts": cb,
        })
    return in_maps


def run(inputs: dict, trace: bool = False, tmpdir: str | None = None):
    nc = build_bass()
    in_maps = _host_inputs(**inputs)
    res = run_bass_kernel_spmd(
        nc, in_maps, core_ids=list(range(N_CORES)), trace=trace, tmpdir=tmpdir,
    )
    outs = [np.asarray(m["out"]).astype(np.float32).reshape(BPC, C, T)
            for m in res.results]
    full = np.concatenate(outs, axis=0)
    return full, res


def kernel(**inputs) -> np.ndarray:
    full, _ = run(inputs, trace=bool(os.environ.get("C2C_TRACE")))
    return full


if __name__ == "__main__":
    # quick single-core numerical check in CoreSim (+ timeline estimate)
    from concourse import bass_interp
    from concourse.timeline_sim import TimelineSim

    rng = np.random.default_rng(0)
    x = rng.standard_normal((BPC, C, T), dtype=np.float32)
    Wq = rng.standard_normal((D, 1)).astype(np.float32)
    bq = rng.standard_normal((D,)).astype(np.float32)
    Wk = rng.standard_normal((D, 1)).astype(np.float32)
    bk = rng.standard_normal((D,)).astype(np.float32)
    alpha = np.float32(0.5)

    nc = build_bass()
    print("timeline estimate:", TimelineSim(nc).simulate(), "ns")

    sim = bass_interp.CoreSim(nc)
    im = _host_inputs(x=np.tile(x, (N_CORES, 1, 1)), Wq=Wq, bq=bq, Wk=Wk, bk=bk,
                      Wv=None, bv=None, alpha=alpha)[0]
    for k, v in im.items():
        sim.tensor(k)[:] = v
    sim.simulate()
    got = np.asarray(sim.tensor("out")).astype(np.float32).reshape(BPC, C, T)

    desc = x.mean(axis=2, keepdims=True)
    q = desc * Wq[:, 0] + bq
    k = desc * Wk[:, 0] + bk
    logits = np.einsum('bcd,bed->bce', q, k) / np.sqrt(D)
    m = logits.max(axis=-1, keepdims=True)
    e = np.exp(logits - m)
    attn = e / e.sum(axis=-1, keepdims=True)
    mixed = np.einsum('bce,bet->bct', attn, x)
    want = x + alpha * mixed
    err = np.abs(got - want)
    rel = np.linalg.norm(got - want) / np.linalg.norm(want)
    print("max abs err:", err.max(), "rel:", rel)


# revision 51
# speedup vs baseline: 1.0874x; 1.0874x over previous
"""Trainium2 Bass kernel for C2C attention.

Computes, for x:(B,C,T)=(32,64,30000) f32:
    desc = mean(x, axis=2)                       # (B,C)
    q = desc*Wq + bq ; k = desc*Wk + bk          # (B,C,D), D=64
    attn = softmax(q @ k^T / sqrt(D))            # (B,C,C)
    out = x + alpha * attn @ x

Sharding: pure data parallel over batch, 4 batches per core on 8 cores.
Each core stacks its 4 batches as 2 "pairs" of 128 SBUF partitions
(2 batches x 64 channels); a block-diagonal 128x128 stationary matrix
computes both batches of a pair in one matmul pass.

Transport is bf16 both ways (host rounds x to bf16; host expands the
bf16 result back to f32) which halves HBM traffic; all computation runs
on device.  Residual + softmax normalization are folded into the matmul
pipeline:

    M^T = diag(sumexp/alpha) + exp(logits)^T    (stationary, bf16)
    out_row_c = (alpha/sumexp_c) * (M x)_c      (scale applied by the
                                                 PSUM->SBUF copy)

Engine plan (GPSIMD cannot touch PSUM, so):
  - DVE: bf16 fold-tree segment reductions (2x DVE mode) + a share of
    the PSUM->SBUF copies once its reduction queue drains
  - ACT: reduction assists on pair0 (activation accum_out), the small
    attention chain, most PSUM->SBUF copies
  - GPSIMD: first-level folds for pair1 reductions, stationary-matrix
    assembly (all SBUF->SBUF)
  - PE: streaming matmul, kept continuously busy so it up-clocks
"""

import os

import numpy as np
import ml_dtypes

import concourse.bass as bass
import concourse.tile as tile
from concourse import bacc, mybir
from concourse.bass_utils import run_bass_kernel_spmd


B, C, T, D = 32, 64, 30000, 64
N_CORES = 8
BPC = B // N_CORES          # batches per core = 4
PAIRS = BPC // 2            # 2
ROWS = BPC * C              # 256 rows of (row, T) per core
SEG = 6000                  # columns per DMA segment (12000B/row descriptors
                            # keep the DMA engines at full ~425GB/s; 6000B
                            # rows measured only ~300GB/s)
NSEG = T // SEG             # 5
CHUNK = 500                 # matmul moving free dim (<=512, fits PSUM bank)
GROUP = 2                   # chunks per PSUM tile (2 banks) -> 1000-col copies
NGRP = SEG // (CHUNK * GROUP)   # 6 groups per segment

F32 = mybir.dt.float32
BF16 = mybir.dt.bfloat16
AX = mybir.AxisListType
AF = mybir.ActivationFunctionType
ALU = mybir.AluOpType

# packed constants layout, one (128, 515) f32 block:
#   [:, 0:128]    identity(128)
#   [:, 128:129]  alpha broadcast
#   [0:2, 129:257]   [Wq/(8T); bq/8 | Wk/T; bk]  (stacked q|k weight rows)
#   [0:2, 257:385]   qk-matmul rhs init: row0 = 0 (sums placeholder), row1 = 1
#   [:, 385:513]  zeros -> attn scratch (off-diagonal blocks must stay 0)
#   [:, 513:514]  ones column (unused)
#   [:, 514:515]  1/alpha broadcast (unused)
#   [:, 515:516]  ln(1/alpha) broadcast (exp bias for the sumexp accum)
CONST_COLS = 516

# pair0 segments whose reduction runs on ACT (rest on DVE — including the
# tail segment, so the last-arriving data is reduced by the faster engine)
P0_ACT_SEGS = {1, 3}
# pair1 segments folded on GPSIMD (rest fully on DVE)
P1_GP_SEGS = (0, 2)
# copy groups assigned to DVE: late groups of pair0, alternating for pair1
P0_DVE_GROUPS = {17, 19, 21, 23, 25, 27, 29}
P1_DVE_GROUPS = {1, 3, 5, 7, 9, 11, 13, 15, 17, 19, 21, 23, 25}


def build_bass() -> bass.Bass:
    nc = bacc.Bacc()

    x = nc.dram_tensor("x", [ROWS, T], BF16, kind="ExternalInput")
    out = nc.dram_tensor("out", [ROWS, T], BF16, kind="ExternalOutput")
    consts_d = nc.dram_tensor("consts", [128, CONST_COLS], F32,
                              kind="ExternalInput")

    with tile.TileContext(nc) as tc, \
            tc.tile_pool(name="consts", bufs=1) as consts, \
            tc.tile_pool(name="pairbuf", bufs=2) as pairbuf, \
            tc.tile_pool(name="fold", bufs=2) as fold, \
            tc.tile_pool(name="xsegs", bufs=PAIRS * NSEG) as xsegs, \
            tc.tile_pool(name="psmm", bufs=3, space="PSUM") as psmm, \
            tc.tile_pool(name="pssm", bufs=2, space="PSUM") as pssm:

        cblk = consts.tile([128, CONST_COLS], F32)
        nc.sync.dma_start(out=cblk, in_=consts_d[:, :])
        ident = cblk[:, 0:128]
        alpha_bc = cblk[:, 128:129]
        wqk2 = cblk[0:2, 129:257]
        rhs_qk = cblk[0:2, 257:385]
        attn = cblk[:, 385:513]
        ln_invalpha = cblk[:, 515:516]
        scratch = consts.tile([128, 1], F32)
        # pre-load the ACT exp table off the critical path
        nc.scalar.activation(out=scratch, in_=alpha_bc, func=AF.Exp)
        # full-size scratch sink for ACT accumulate-reductions
        accsink = consts.tile([128, SEG], BF16)

        xs = [[None] * NSEG for _ in range(PAIRS)]
        partials = [None] * PAIRS
        lhsT = [None] * PAIRS
        arec = [None] * PAIRS
        sums = [None] * PAIRS
        sumexp = [None] * PAIRS
        diags = [None] * PAIRS

        def emit_load(p, s):
            xt = xsegs.tile([128, SEG], BF16, tag="xseg")
            xs[p][s] = xt
            nc.sync.dma_start(
                out=xt,
                in_=x[p * 128:(p + 1) * 128, s * SEG:(s + 1) * SEG],
            )

        def emit_reduce_dve(p, s):
            xt = xs[p][s]
            h = fold.tile([128, SEG // 2], BF16, tag="h")
            nc.vector.tensor_add(out=h, in0=xt[:, 0:SEG // 2],
                                 in1=xt[:, SEG // 2:SEG])
            q = fold.tile([128, SEG // 4], BF16, tag="q")
            nc.vector.tensor_add(out=q, in0=h[:, 0:SEG // 4],
                                 in1=h[:, SEG // 4:SEG // 2])
            w = fold.tile([128, SEG // 8], BF16, tag="w")
            nc.vector.tensor_add(out=w, in0=q[:, 0:SEG // 8],
                                 in1=q[:, SEG // 8:SEG // 4])
            nc.vector.reduce_sum(out=partials[p][:, s:s + 1], in_=w, axis=AX.X)

        def emit_reduce_act(p, s):
            # ACT-assisted reduction: copy into a scratch sink, accumulate
            # the row sum as a side effect
            nc.scalar.activation(out=accsink, in_=xs[p][s], func=AF.Copy,
                                 accum_out=partials[p][:, s:s + 1])

        gp_folds = {}

        def emit_gp_fold(p, s):
            # both fold levels on GPSIMD (SBUF->SBUF); DVE only reduces
            xt = xs[p][s]
            gh = fold.tile([128, SEG // 2], BF16, tag="gh")
            nc.gpsimd.tensor_add(out=gh, in0=xt[:, 0:SEG // 2],
                                 in1=xt[:, SEG // 2:SEG])
            gq = fold.tile([128, SEG // 4], BF16, tag="gq")
            nc.gpsimd.tensor_add(out=gq, in0=gh[:, 0:SEG // 4],
                                 in1=gh[:, SEG // 4:SEG // 2])
            gp_folds[(p, s)] = gq

        def emit_dve_finish(p, s):
            gq = gp_folds[(p, s)]
            w = fold.tile([128, SEG // 8], BF16, tag="w")
            nc.vector.tensor_add(out=w, in0=gq[:, 0:SEG // 8],
                                 in1=gq[:, SEG // 8:SEG // 4])
            nc.vector.reduce_sum(out=partials[p][:, s:s + 1], in_=w, axis=AX.X)

        def alloc_partials(p):
            part = pairbuf.tile([128, NSEG], F32, tag="partial")
            partials[p] = part

        def emit_sums(p):
            sm = pairbuf.tile([128, 1], F32, tag="sums")
            nc.vector.reduce_sum(out=sm, in_=partials[p], axis=AX.X)
            sums[p] = sm

        def emit_smalls_head(p):
            # transpose total sums to a row: (1,128)
            srow_ps = pssm.tile([1, 128], F32, tag="ps_small")
            nc.tensor.transpose(out=srow_ps, in_=sums[p], identity=ident)
            nc.scalar.copy(out=rhs_qk[0:1, :], in_=srow_ps)
            # [qT; kT] = [wq'|wk']^T @ [sums_row; ones] : (128, 2C)
            qk_ps = pssm.tile([128, 2 * C], F32, tag="ps_small")
            nc.tensor.matmul(out=qk_ps, lhsT=wqk2, rhs=rhs_qk,
                             start=True, stop=True)
            qT = pairbuf.tile([D, 2 * C], F32, tag="qT")
            nc.scalar.copy(out=qT, in_=qk_ps[0:64, :])
            kT = pairbuf.tile([D, 2 * C], F32, tag="kT")
            nc.scalar.copy(out=kT, in_=qk_ps[64:128, :])
            # transposed logits lgT[e,c] = k_e . q_c, and plain logits
            # lg[c,e] (back-to-back PE ops, no extra hop)
            lg_ps = pssm.tile([128, 128], F32, tag="ps_small")
            nc.tensor.matmul(out=lg_ps, lhsT=qT, rhs=kT, start=True, stop=True)
            lgT_ps = pssm.tile([128, 128], F32, tag="ps_small")
            nc.tensor.matmul(out=lgT_ps, lhsT=kT, rhs=qT, start=True, stop=True)
            # exp of each diagonal block into the zeroed attn scratch; the
            # second (throwaway-output) exp over the plain logits accumulates
            # se_c = sum_e exp(lg[c,e] + ln(1/alpha)) = sumexp_c / alpha, so
            # reciprocal(se) = alpha/sumexp — exactly the copy scale
            se = pairbuf.tile([128, 1], F32, tag="sumexp")
            for h in range(2):
                r = slice(h * 64, h * 64 + 64)
                nc.scalar.activation(out=attn[r, r], in_=lgT_ps[r, r],
                                     func=AF.Exp)
                nc.scalar.activation(out=accsink[r, 0:64], in_=lg_ps[r, r],
                                     func=AF.Exp, bias=ln_invalpha[r, :],
                                     accum_out=se[r, :])
            sumexp[p] = se
            # diagonal part of the stationary matrix: diag(sumexp/alpha)
            diag = pairbuf.tile([128, 128], F32, tag="diag")
            nc.scalar.activation(out=diag, in_=ident, func=AF.Copy, scale=se)
            diags[p] = diag

        def emit_lt_dve(p):
            # stationary matrix M^T = diag(sumexp/alpha) + exp(logits)^T
            lt = pairbuf.tile([128, 128], BF16, tag="lhsT")
            nc.vector.tensor_add(out=lt, in0=diags[p], in1=attn)
            lhsT[p] = lt

        def emit_recip(p):
            # alpha/sumexp scale for the PSUM->SBUF copies
            ar = pairbuf.tile([128, 1], F32, tag="arec")
            nc.vector.reciprocal(out=ar, in_=sumexp[p])
            arec[p] = ar

        def emit_compute(p, segs, dve_groups, split_last=False):
            for s in segs:
                xt = xs[p][s]
                for g in range(NGRP):
                    gi = s * NGRP + g
                    mm = psmm.tile([128, GROUP, 512], F32, tag="mm")
                    base = g * GROUP * CHUNK
                    for j in range(GROUP):
                        nc.tensor.matmul(
                            out=mm[:, j, 0:CHUNK],
                            lhsT=lhsT[p],
                            rhs=xt[:, base + j * CHUNK: base + (j + 1) * CHUNK],
                            start=True, stop=True,
                        )
                    dst = xt[:, base: base + GROUP * CHUNK].rearrange(
                        "p (a c) -> p a c", a=GROUP)
                    if gi in dve_groups:
                        nc.vector.tensor_scalar(out=dst, in0=mm[:, :, 0:CHUNK],
                                                scalar1=arec[p], scalar2=None,
                                                op0=ALU.mult)
                    else:
                        nc.scalar.activation(out=dst, in_=mm[:, :, 0:CHUNK],
                                             func=AF.Copy, scale=arec[p])
                    if split_last and s == segs[-1] and g % 2 == 1:
                        # final segment: drain per pair-of-groups so the last
                        # DMA is small and the tail is short
                        pb = (g - 1) * GROUP * CHUNK
                        nc.sync.dma_start(
                            out=out[p * 128:(p + 1) * 128,
                                    s * SEG + pb: s * SEG + pb
                                    + 2 * GROUP * CHUNK],
                            in_=xt[:, pb: pb + 2 * GROUP * CHUNK],
                        )
                if not (split_last and s == segs[-1]):
                    nc.sync.dma_start(
                        out=out[p * 128:(p + 1) * 128,
                                s * SEG:(s + 1) * SEG],
                        in_=xt,
                    )

        # ---- schedule ----
        alloc_partials(0)
        alloc_partials(1)
        for s in range(NSEG):
            emit_load(0, s)
            if s in P0_ACT_SEGS:
                emit_reduce_act(0, s)
            else:
                emit_reduce_dve(0, s)
        for s in range(NSEG):
            emit_load(1, s)
        emit_sums(0)
        emit_smalls_head(0)
        # pair1 reductions start while the pair0 attention chain finishes;
        # GPSIMD takes the first fold level on early segments so DVE can
        # also pick up part of the pair0 copy load later.  The pair0
        # stationary-matrix add + reciprocal slot into the DVE stream
        # between segment finishes, right when their inputs appear.
        # GPSIMD folds its segments sequentially from first arrival; the DVE
        # stream runs its own independent fold-trees interleaved with the
        # pair0 stationary-matrix add, the copy-scale reciprocal, and the
        # finish-reduces of GPSIMD's segments (ordered so DVE never
        # head-of-line blocks on the slower GPSIMD queue)
        for s in P1_GP_SEGS:
            emit_gp_fold(1, s)
        emit_reduce_dve(1, 1)
        emit_lt_dve(0)
        emit_recip(0)
        emit_reduce_dve(1, 3)
        emit_dve_finish(1, 0)
        emit_reduce_dve(1, 4)
        emit_dve_finish(1, 2)
        emit_sums(1)
        emit_compute(0, range(0, 4), P0_DVE_GROUPS)
        emit_smalls_head(1)
        emit_recip(1)
        emit_lt_dve(1)
        emit_compute(0, range(4, NSEG), P0_DVE_GROUPS)
        emit_compute(1, range(NSEG), P1_DVE_GROUPS, split_last=True)

    nc.compile()
    return nc


def _host_inputs(x, Wq, bq, Wk, bk, Wv, bv, alpha):
    """Build per-core input maps. Scale folding:
    logits[c,e] = (q[c]/8) . k[e],  q/8 = (Wq/(8T))*sums + bq/8, k = (Wk/T)*sums + bk
    """
    x = np.asarray(x, dtype=np.float32).astype(ml_dtypes.bfloat16)
    cb = np.zeros((128, CONST_COLS), dtype=np.float32)
    cb[:, 0:128] = np.eye(128, dtype=np.float32)
    cb[:, 128] = np.float32(alpha)
    cb[0, 129:193] = np.asarray(Wq)[:, 0] / (8.0 * T)
    cb[1, 129:193] = np.asarray(bq) / 8.0
    cb[0, 193:257] = np.asarray(Wk)[:, 0] / T
    cb[1, 193:257] = np.asarray(bk)
    cb[1, 257:385] = 1.0
    cb[:, 513] = 1.0
    cb[:, 514] = 1.0 / np.float32(alpha)
    cb[:, 515] = np.log(1.0 / np.float64(alpha)).astype(np.float32)
    in_maps = []
    for c in range(N_CORES):
        shard = x[c * BPC:(c + 1) * BPC].reshape(ROWS, T)
        in_maps.append({
            "x": np.ascontiguousarray(shard),
            "consts": cb,
        })
    return in_maps


def run(inputs: dict, trace: bool = False, tmpdir: str | None = None):
    nc = build_bass()
    in_maps = _host_inputs(**inputs)
    res = run_bass_kernel_spmd(
        nc, in_maps, core_ids=list(range(N_CORES)), trace=trace, tmpdir=tmpdir,
    )
    outs = [np.asarray(m["out"]).astype(np.float32).reshape(BPC, C, T)
            for m in res.results]
    full = np.concatenate(outs, axis=0)
    return full, res


def kernel(**inputs) -> np.ndarray:
    full, _ = run(inputs, trace=bool(os.environ.get("C2C_TRACE")))
    return full


if __name__ == "__main__":
    # quick single-core numerical check in CoreSim (+ timeline estimate)
    from concourse import bass_interp
    from concourse.timeline_sim import TimelineSim

    rng = np.random.default_rng(0)
    x = rng.standard_normal((BPC, C, T), dtype=np.float32)
    Wq = rng.standard_normal((D, 1)).astype(np.float32)
    bq = rng.standard_normal((D,)).astype(np.float32)
    Wk = rng.standard_normal((D, 1)).astype(np.float32)
    bk = rng.standard_normal((D,)).astype(np.float32)
    alpha = np.float32(0.5)

    nc = build_bass()
    print("timeline estimate:", TimelineSim(nc).simulate(), "ns")

    sim = bass_interp.CoreSim(nc)
    im = _host_inputs(x=np.tile(x, (N_CORES, 1, 1)), Wq=Wq, bq=bq, Wk=Wk, bk=bk,
                      Wv=None, bv=None, alpha=alpha)[0]
    for k, v in im.items():
        sim.tensor(k)[:] = v
    sim.simulate()
    got = np.asarray(sim.tensor("out")).astype(np.float32).reshape(BPC, C, T)

    desc = x.mean(axis=2, keepdims=True)
    q = desc * Wq[:, 0] + bq
    k = desc * Wk[:, 0] + bk
    logits = np.einsum('bcd,bed->bce', q, k) / np.sqrt(D)
    m = logits.max(axis=-1, keepdims=True)
    e = np.exp(logits - m)
    attn = e / e.sum(axis=-1, keepdims=True)
    mixed = np.einsum('bce,bet->bct', attn, x)
    want = x + alpha * mixed
    err = np.abs(got - want)
    rel = np.linalg.norm(got - want) / np.linalg.norm(want)
    print("max abs err:", err.max(), "rel:", rel)
